# revision 10
# baseline (speedup 1.0000x reference)
"""Trainium2 Bass kernel for nn_AudioPreprocessor (binaural STFT features).

Contract: kernel(**inputs) takes the FULL unsharded inputs (numpy) and
returns the full [8, 6, 64, 1001] float32 output. Internally: data-parallel
over batch across 8 NeuronCores (one batch per core, no collectives).

Pipeline per core (batch b):
  A) STFT of L/R channels as DFT-matmuls (f32r), accumulate in PSUM over
     16 contraction chunks; derive pow_L, pow_R, csd_r, csd_i into SBUF
     (bf16, resident [128, 9, 1001] per array, freq-padded 1025->1152).
  M) mel_fb matmuls -> mel_L/R, mel_csd_r/i [64, 1001].
  B) per direction tile (18 x 128 dirs): n1/n2/corr_un matmuls (bf16
     weights streamed), normalize corr = corr_un*rsqrt(n1*n2+1e-8),
     scatter into azimuth bins via 0/1 matmul accumulated in PSUM.
  C) finalize ch0..ch5, DMA out [6, 64, 1001].
"""
import sys
import os

_RL = "/opt/trn_rl_repo"
if _RL not in sys.path:
    sys.path.insert(0, _RL)

import numpy as np
import ml_dtypes
from contextlib import ExitStack

import concourse.bass as bass
import concourse.mybir as mybir
import concourse.tile as tile
from concourse.bass_utils import run_bass_kernel_spmd

# ---------------- problem constants (hardcoded) ----------------
B = 8
NS = 960000
NFFT = 2048
HOP = 960
F = 1025
FP = 1152            # padded freq dim: 9 * 128
NFT = FP // 128      # 9 freq tiles
NCH = 16             # 2048 / 128 contraction chunks
NM = 64
ND = 2304
NDT = ND // 128      # 18 direction tiles
T = 1001
TC = 1002           # computed columns (f32r matmul needs even N); last col discarded
EPS = 1e-8
PADNS = NS + NFFT + HOP   # 963008 (one extra hop so we can compute T_C=1002 columns)
K_LOG = 10.0 / np.log(10.0)

T_PASSES = [(0, 334), (334, 334), (668, 334)]   # phase A (N>=256, even, for f32r)
T_HALVES = [(0, 512), (512, 490)]               # matmul N<=512 splits of TC

F32 = mybir.dt.float32
F32R = mybir.dt.float32r
BF16 = mybir.dt.bfloat16
AF = mybir.ActivationFunctionType

_NC_CACHE = {}


def _budget(inst) -> int:
    return 1


def fix_sync_waits(nc):
    """Split per-instruction sync waits: this walrus build allows only ONE
    sync wait per instruction; hoist extras onto NoOps just before, on the
    same engine queue (engine streams execute in block order)."""
    counter = 0
    for f in nc.m.functions:
        for bb in f.blocks:
            new_insts = []
            changed = False
            for inst in bb.instructions:
                si = inst.sync_info
                waits = list(si.on_wait) if si is not None and si.on_wait else []
                budget = _budget(inst)
                if len(waits) > budget:
                    keep = waits[len(waits) - budget:]
                    excess = waits[: len(waits) - budget]
                    for wt in excess:
                        counter += 1
                        nop = mybir.InstNoOp(name=f"I-waitfix-{counter}")
                        nop.engine = inst.engine
                        nop.sync_info = mybir.SyncInfo(on_wait=[wt], on_update=[])
                        new_insts.append(nop)
                    inst.sync_info = mybir.SyncInfo(
                        on_wait=keep,
                        on_update=list(si.on_update) if si.on_update else [],
                    )
                    changed = True
                new_insts.append(inst)
            if changed:
                bb.instructions = new_insts
    return nc


def build_nc():
    nc = bass.Bass()

    xpad = nc.declare_dram_parameter("xpad", [2, PADNS], F32, isOutput=False)
    dftc = nc.declare_dram_parameter("dftc", [NFT, 128, NCH, 128], F32, isOutput=False)
    dfts = nc.declare_dram_parameter("dfts", [NFT, 128, NCH, 128], F32, isOutput=False)
    wwr = nc.declare_dram_parameter("wwr", [NDT, 128, NFT, 128], BF16, isOutput=False)
    wwi = nc.declare_dram_parameter("wwi", [NDT, 128, NFT, 128], BF16, isOutput=False)
    whr = nc.declare_dram_parameter("whr", [NDT, 128, NFT, 128], BF16, isOutput=False)
    whl = nc.declare_dram_parameter("whl", [NDT, 128, NFT, 128], BF16, isOutput=False)
    melw = nc.declare_dram_parameter("melw", [128, NFT, NM], BF16, isOutput=False)
    melwf = nc.declare_dram_parameter("melwf", [128, NFT, NM], F32, isOutput=False)
    sTw = nc.declare_dram_parameter("sTw", [NDT, 128, NM], BF16, isOutput=False)
    rcnt = nc.declare_dram_parameter("rcnt", [NM, 1], F32, isOutput=False)
    out = nc.declare_dram_parameter("out", [6, NM, T], F32, isOutput=True)

    with tile.TileContext(nc) as tc, ExitStack() as ctx:
        # resident pow/csd arrays [128, 9, 1001] bf16
        resid = ctx.enter_context(tc.tile_pool(name="resid", bufs=1))
        powL = resid.tile([128, NFT, TC], BF16, tag="powL")
        powR = resid.tile([128, NFT, TC], BF16, tag="powR")
        csdR = resid.tile([128, NFT, TC], BF16, tag="csdR")
        csdI = resid.tile([128, NFT, TC], BF16, tag="csdI")
        # mel/channel tiles [64, 1001] f32 + consts
        mels = ctx.enter_context(tc.tile_pool(name="mels", bufs=1))
        melL_s = mels.tile([NM, TC], F32, tag="melL")
        melR_s = mels.tile([NM, TC], F32, tag="melR")
        melr_s = mels.tile([NM, TC], F32, tag="melr")
        meli_s = mels.tile([NM, TC], F32, tag="meli")
        eb128 = mels.tile([128, 1], F32, tag="eb128")
        rcnt_s = mels.tile([NM, 1], F32, tag="rcnt")
        nc.vector.memset(eb128, EPS)
        nc.sync.dma_start(out=rcnt_s, in_=rcnt[:, :])
        eb64 = eb128[:NM, :]
        # f32 csd staging in DRAM (for high-precision mel_csd path: ch3/ch4)
        dpool = ctx.enter_context(tc.tile_pool(name="csdd", bufs=1, space="DRAM"))
        csdr32 = dpool.tile([128, NFT, TC], F32, tag="csdr32")
        csdi32 = dpool.tile([128, NFT, TC], F32, tag="csdi32")

        # ---------------- Phase A: STFT + pow/csd ----------------
        with tc.tile_pool(name="frames", bufs=1) as fpool, \
             tc.tile_pool(name="dftw", bufs=2) as wpool, \
             tc.tile_pool(name="patmp", bufs=2) as tpool, \
             tc.tile_pool(name="psumA", bufs=2, space="PSUM") as psA:
            for (t0, tcnt) in T_PASSES:
                ftiles = {}
                for ch in range(2):
                    for c in range(NCH):
                        ft = fpool.tile([128, tcnt], F32R, tag=f"f{ch}_{c}")
                        src = bass.AP(
                            tensor=xpad,
                            offset=ch * PADNS + HOP * t0 + 128 * c,
                            ap=[[1, 128], [HOP, tcnt]],
                        ).bitcast(F32R)
                        nc.sync.dma_start(out=ft, in_=src)
                        ftiles[(ch, c)] = ft
                for fi in range(NFT):
                    wc = wpool.tile([128, NCH, 128], F32R, tag="wc")
                    nc.sync.dma_start(out=wc, in_=dftc[fi].bitcast(F32R))
                    ws = wpool.tile([128, NCH, 128], F32R, tag="ws")
                    nc.sync.dma_start(out=ws, in_=dfts[fi].bitcast(F32R))
                    plr = psA.tile([128, tcnt], F32, tag="plr")
                    pli = psA.tile([128, tcnt], F32, tag="pli")
                    prr = psA.tile([128, tcnt], F32, tag="prr")
                    pri = psA.tile([128, tcnt], F32, tag="pri")
                    for c in range(NCH):
                        st, sp = (c == 0), (c == NCH - 1)
                        nc.tensor.matmul(plr, wc[:, c, :], ftiles[(0, c)],
                                         start=st, stop=sp, skip_group_check=True)
                        nc.tensor.matmul(prr, wc[:, c, :], ftiles[(1, c)],
                                         start=st, stop=sp, skip_group_check=True)
                        nc.tensor.matmul(pli, ws[:, c, :], ftiles[(0, c)],
                                         start=st, stop=sp, skip_group_check=True)
                        nc.tensor.matmul(pri, ws[:, c, :], ftiles[(1, c)],
                                         start=st, stop=sp, skip_group_check=True)
                    # derive pow/csd for this (pass, fi)
                    tsl = slice(t0, t0 + tcnt)
                    t1 = tpool.tile([128, tcnt], F32, tag="t1")
                    t2 = tpool.tile([128, tcnt], F32, tag="t2")
                    t3 = tpool.tile([128, tcnt], F32, tag="t3")
                    t4 = tpool.tile([128, tcnt], F32, tag="t4")
                    a_r = tpool.tile([128, tcnt], F32, tag="a_r")
                    a_i = tpool.tile([128, tcnt], F32, tag="a_i")
                    nc.scalar.copy(a_r, plr)
                    nc.scalar.copy(a_i, pli)
                    nc.scalar.activation(t1, plr, AF.Square)
                    nc.scalar.activation(t2, pli, AF.Square)
                    nc.scalar.activation(t3, prr, AF.Square)
                    nc.scalar.activation(t4, pri, AF.Square)
                    nc.vector.tensor_add(powL[:, fi, tsl], t1, t2)
                    nc.vector.tensor_add(powR[:, fi, tsl], t3, t4)
                    m1 = tpool.tile([128, tcnt], F32, tag="m1")
                    m2 = tpool.tile([128, tcnt], F32, tag="m2")
                    nc.vector.tensor_mul(m1, a_r, prr)
                    nc.vector.tensor_mul(m2, a_i, pri)
                    csdf = tpool.tile([128, tcnt], F32, tag="csdf")
                    nc.vector.tensor_add(csdf, m1, m2)
                    nc.scalar.copy(csdR[:, fi, tsl], csdf)
                    nc.sync.dma_start(out=csdr32[:, fi, tsl], in_=csdf)
                    m3 = tpool.tile([128, tcnt], F32, tag="m3")
                    m4 = tpool.tile([128, tcnt], F32, tag="m4")
                    nc.vector.tensor_mul(m3, a_i, prr)
                    nc.vector.tensor_mul(m4, a_r, pri)
                    csif = tpool.tile([128, tcnt], F32, tag="csif")
                    nc.vector.tensor_sub(csif, m3, m4)
                    nc.scalar.copy(csdI[:, fi, tsl], csif)
                    nc.sync.dma_start(out=csdi32[:, fi, tsl], in_=csif)

        # ---------------- Mel phase ----------------
        with tc.tile_pool(name="melwp", bufs=1) as mwp, \
             tc.tile_pool(name="melrhs", bufs=4) as mrp, \
             tc.tile_pool(name="psumM", bufs=1, space="PSUM") as psM:
            melw_s = mwp.tile([128, NFT, NM], BF16, tag="melw")
            nc.sync.dma_start(out=melw_s, in_=melw[:, :, :])
            melwf_s = mwp.tile([128, NFT, NM], F32, tag="melwf")
            nc.sync.dma_start(out=melwf_s, in_=melwf[:, :, :])
            # pow -> mel in bf16 (log-scale outputs are tolerant)
            for (name, rhs, dst) in (("mL", powL, melL_s), ("mR", powR, melR_s)):
                pm = psM.tile([NM, TC], F32, tag=f"pm_{name}")
                for fi in range(NFT):
                    for (h0, hc) in T_HALVES:
                        nc.tensor.matmul(
                            pm[:, h0:h0 + hc], melw_s[:, fi, :],
                            rhs[:, fi, h0:h0 + hc],
                            start=(fi == 0), stop=(fi == NFT - 1),
                            skip_group_check=True)
                nc.scalar.copy(dst, pm)
            # csd -> mel in full fp32 (ch3/ch4 are ill-conditioned where
            # mel_csd cancels; bf16 there saturates the direction error)
            for (name, src, dst) in (("mr", csdr32, melr_s), ("mi", csdi32, meli_s)):
                pm = psM.tile([NM, TC], F32, tag=f"pm_{name}")
                for fi in range(NFT):
                    for (h0, hc) in T_HALVES:
                        rt = mrp.tile([128, hc], F32, tag=f"rt{hc}")
                        nc.sync.dma_start(out=rt, in_=src[:, fi, h0:h0 + hc])
                        nc.tensor.matmul(
                            pm[:, h0:h0 + hc], melwf_s[:, fi, :], rt,
                            start=(fi == 0), stop=(fi == NFT - 1),
                            skip_group_check=True)
                nc.scalar.copy(dst, pm)

        # ---------------- Phase B: corr + scatter ----------------
        with tc.tile_pool(name="wB", bufs=2) as wBp, \
             tc.tile_pool(name="pbt", bufs=2) as pbt, \
             tc.tile_pool(name="psumB", bufs=1, space="PSUM") as psB:
            ch5p = psB.tile([NM, TC], F32, tag="ch5")
            for dj in range(NDT):
                whr_t = wBp.tile([128, NFT, 128], BF16, tag="whr")
                nc.sync.dma_start(out=whr_t, in_=whr[dj])
                whl_t = wBp.tile([128, NFT, 128], BF16, tag="whl")
                nc.sync.dma_start(out=whl_t, in_=whl[dj])
                wwr_t = wBp.tile([128, NFT, 128], BF16, tag="wwr")
                nc.sync.dma_start(out=wwr_t, in_=wwr[dj])
                wwi_t = wBp.tile([128, NFT, 128], BF16, tag="wwi")
                nc.sync.dma_start(out=wwi_t, in_=wwi[dj])
                sT_t = wBp.tile([128, NM], BF16, tag="sT")
                nc.sync.dma_start(out=sT_t, in_=sTw[dj])

                n1 = psB.tile([128, TC], F32, tag="n1")
                n2 = psB.tile([128, TC], F32, tag="n2")
                cu = psB.tile([128, TC], F32, tag="cu")
                for fi in range(NFT):
                    st, sp = (fi == 0), (fi == NFT - 1)
                    for (h0, hc) in T_HALVES:
                        hs = slice(h0, h0 + hc)
                        nc.tensor.matmul(n1[:, hs], whr_t[:, fi, :],
                                         powL[:, fi, hs], start=st, stop=sp,
                                         skip_group_check=True)
                        nc.tensor.matmul(n2[:, hs], whl_t[:, fi, :],
                                         powR[:, fi, hs], start=st, stop=sp,
                                         skip_group_check=True)
                        nc.tensor.matmul(cu[:, hs], wwr_t[:, fi, :],
                                         csdR[:, fi, hs], start=st, stop=False,
                                         skip_group_check=True)
                        nc.tensor.matmul(cu[:, hs], wwi_t[:, fi, :],
                                         csdI[:, fi, hs], start=False, stop=sp,
                                         skip_group_check=True)
                cn1 = pbt.tile([128, TC], F32, tag="cn1")
                nc.scalar.copy(cn1, n1)
                den = pbt.tile([128, TC], F32, tag="den")
                nc.vector.tensor_mul(den, cn1, n2)
                sq = pbt.tile([128, TC], F32, tag="sq")
                nc.scalar.activation(sq, den, AF.Sqrt, bias=eb128)
                rden = pbt.tile([128, TC], F32, tag="rden")
                nc.vector.reciprocal(rden, sq)
                corr = pbt.tile([128, TC], BF16, tag="corr")
                nc.vector.tensor_mul(corr, cu, rden)
                for (h0, hc) in T_HALVES:
                    hs = slice(h0, h0 + hc)
                    nc.tensor.matmul(ch5p[:, hs], sT_t, corr[:, hs],
                                     start=(dj == 0), stop=(dj == NDT - 1),
                                     skip_group_check=True)

            # ---------------- Phase C: finalize ----------------
            with tc.tile_pool(name="fint", bufs=1) as fin:
                # ch0 / ch1
                u0 = fin.tile([NM, TC], F32, tag="u0")
                nc.scalar.activation(u0, melL_s, AF.Ln, bias=eb64)
                c0 = fin.tile([NM, TC], F32, tag="c0")
                nc.scalar.mul(c0, u0, K_LOG)
                nc.sync.dma_start(out=out[0], in_=c0[:, :T])
                u1 = fin.tile([NM, TC], F32, tag="u1")
                nc.scalar.activation(u1, melR_s, AF.Ln, bias=eb64)
                c1 = fin.tile([NM, TC], F32, tag="c1")
                nc.scalar.mul(c1, u1, K_LOG)
                nc.sync.dma_start(out=out[1], in_=c1[:, :T])
                # ch2 = K*ln(melL/(melR+eps) + eps)
                v2 = fin.tile([NM, TC], F32, tag="v2")
                nc.scalar.activation(v2, melR_s, AF.Identity, bias=eb64)
                r2 = fin.tile([NM, TC], F32, tag="r2")
                nc.vector.reciprocal(r2, v2)
                w2 = fin.tile([NM, TC], F32, tag="w2")
                nc.vector.tensor_mul(w2, melL_s, r2)
                u2 = fin.tile([NM, TC], F32, tag="u2")
                nc.scalar.activation(u2, w2, AF.Ln, bias=eb64)
                c2 = fin.tile([NM, TC], F32, tag="c2")
                nc.scalar.mul(c2, u2, K_LOG)
                nc.sync.dma_start(out=out[2], in_=c2[:, :T])
                # ch3/ch4
                s1 = fin.tile([NM, TC], F32, tag="s1")
                nc.scalar.activation(s1, melr_s, AF.Square)
                s2 = fin.tile([NM, TC], F32, tag="s2")
                nc.scalar.activation(s2, meli_s, AF.Square)
                ss = fin.tile([NM, TC], F32, tag="ss")
                nc.vector.tensor_add(ss, s1, s2)
                nrm = fin.tile([NM, TC], F32, tag="nrm")
                nc.scalar.activation(nrm, ss, AF.Sqrt, bias=eb64)
                rn = fin.tile([NM, TC], F32, tag="rn")
                nc.vector.reciprocal(rn, nrm)
                c3 = fin.tile([NM, TC], F32, tag="c3")
                nc.vector.tensor_mul(c3, meli_s, rn)
                nc.sync.dma_start(out=out[3], in_=c3[:, :T])
                c4 = fin.tile([NM, TC], F32, tag="c4")
                nc.vector.tensor_mul(c4, melr_s, rn)
                nc.sync.dma_start(out=out[4], in_=c4[:, :T])
                # ch5 = ch5p * rcnt (per-partition scalar)
                c5 = fin.tile([NM, TC], F32, tag="c5")
                nc.vector.tensor_scalar_mul(c5, ch5p, rcnt_s)
                nc.sync.dma_start(out=out[5], in_=c5[:, :T])

    fix_sync_waits(nc)
    return nc


def _host_prep(inputs):
    wav = np.asarray(inputs["waveform"], dtype=np.float32)          # [8,2,NS]
    W_real = np.asarray(inputs["W_real"], dtype=np.float32)         # [ND,F]
    W_imag = np.asarray(inputs["W_imag"], dtype=np.float32)
    norm_hr = np.asarray(inputs["norm_hr_sq"], dtype=np.float32)
    norm_hl = np.asarray(inputs["norm_hl_sq"], dtype=np.float32)
    az = np.asarray(inputs["az_bin_idx"]).astype(np.int64)          # [ND]
    win = np.asarray(inputs["window"], dtype=np.float32)            # [NFFT]
    mel_fb = np.asarray(inputs["mel_fb"], dtype=np.float32)         # [NM,F]

    xpad = np.pad(wav, ((0, 0), (0, 0), (NFFT // 2, NFFT // 2 + HOP)))  # [8,2,PADNS]

    n = np.arange(NFFT, dtype=np.float64)[:, None]
    k = np.arange(F, dtype=np.float64)[None, :]
    ang = 2.0 * np.pi * n * k / NFFT
    dftc = np.zeros((NFFT, FP), dtype=np.float32)
    dfts = np.zeros((NFFT, FP), dtype=np.float32)
    dftc[:, :F] = (np.cos(ang) * win[:, None]).astype(np.float32)
    dfts[:, :F] = (-np.sin(ang) * win[:, None]).astype(np.float32)

    def tile_dft(a):  # [NFFT, FP] -> [NFT, 128, NCH, 128]
        return np.ascontiguousarray(
            a.reshape(NCH, 128, NFT, 128).transpose(2, 1, 0, 3))

    def tile_w(mat):  # [ND, F] -> lhsT tiled [NDT, 128, NFT, 128] bf16
        t = np.zeros((FP, ND), dtype=np.float32)
        t[:F] = mat.T
        return np.ascontiguousarray(
            t.reshape(NFT, 128, NDT, 128).transpose(2, 1, 0, 3)
        ).astype(ml_dtypes.bfloat16)

    wwr = tile_w(W_real)
    wwi = tile_w(-W_imag)
    whr = tile_w(norm_hr)
    whl = tile_w(norm_hl)

    melT = np.zeros((FP, NM), dtype=np.float32)
    melT[:F] = mel_fb.T
    melwf = np.ascontiguousarray(melT.reshape(NFT, 128, NM).transpose(1, 0, 2))
    melw = melwf.astype(ml_dtypes.bfloat16)

    cnt = np.bincount(az, minlength=NM).astype(np.float32)
    S01 = (az[:, None] == np.arange(NM)[None, :]).astype(np.float32)  # [ND,NM]
    sTw = np.ascontiguousarray(S01.reshape(NDT, 128, NM)).astype(ml_dtypes.bfloat16)
    rcnt = (1.0 / (cnt + EPS)).astype(np.float32).reshape(NM, 1)

    shared = {
        "dftc": tile_dft(dftc), "dfts": tile_dft(dfts),
        "wwr": wwr, "wwi": wwi, "whr": whr, "whl": whl,
        "melw": melw, "melwf": melwf, "sTw": sTw, "rcnt": rcnt,
    }
    in_maps = []
    for b in range(B):
        m = dict(shared)
        m["xpad"] = np.ascontiguousarray(xpad[b])
        in_maps.append(m)
    return in_maps


def kernel(**inputs) -> np.ndarray:
    if "nc" not in _NC_CACHE:
        _NC_CACHE["nc"] = build_nc()
    nc = _NC_CACHE["nc"]
    in_maps = _host_prep(inputs)
    res = run_bass_kernel_spmd(nc, in_maps, core_ids=list(range(B)))
    out = np.stack([np.asarray(res.results[i]["out"]) for i in range(B)])
    return out.astype(np.float32)


# revision 14
# speedup vs baseline: 3.7735x; 3.7735x over previous
"""Trainium2 Bass kernel for nn_AudioPreprocessor (binaural STFT features).

Contract: kernel(**inputs) takes the FULL unsharded inputs (numpy) and
returns the full [8, 6, 64, 1001] float32 output. Internally: data-parallel
over batch across 8 NeuronCores (one batch per core, no collectives).

Pipeline per core (batch b):
  A) STFT of L/R channels as DFT-matmuls (f32r), accumulate in PSUM over
     16 contraction chunks; derive pow_L, pow_R, csd_r, csd_i into SBUF
     (bf16, resident [128, 9, 1001] per array, freq-padded 1025->1152).
  M) mel_fb matmuls -> mel_L/R, mel_csd_r/i [64, 1001].
  B) per direction tile (18 x 128 dirs): n1/n2/corr_un matmuls (bf16
     weights streamed), normalize corr = corr_un*rsqrt(n1*n2+1e-8),
     scatter into azimuth bins via 0/1 matmul accumulated in PSUM.
  C) finalize ch0..ch5, DMA out [6, 64, 1001].
"""
import sys
import os

_RL = "/opt/trn_rl_repo"
if _RL not in sys.path:
    sys.path.insert(0, _RL)

import numpy as np
import ml_dtypes
from contextlib import ExitStack

import concourse.bass as bass
import concourse.mybir as mybir
import concourse.tile as tile
from concourse.bass_utils import run_bass_kernel_spmd

# ---------------- problem constants (hardcoded) ----------------
B = 8
NS = 960000
NFFT = 2048
HOP = 960
F = 1025
FP = 1152            # padded freq dim: 9 * 128
NFT = FP // 128      # 9 freq tiles
NCH = 16             # 2048 / 128 contraction chunks
NM = 64
ND = 2304
NDT = ND // 128      # 18 direction tiles
T = 1001
TC = 1002           # computed columns (f32r matmul needs even N); last col discarded
EPS = 1e-8
PADNS = NS + NFFT + HOP   # 963008 (one extra hop so we can compute T_C=1002 columns)
K_LOG = 10.0 / np.log(10.0)

T_PASSES = [(0, 334), (334, 334), (668, 334)]   # phase A (N>=256, even, for f32r)
T_HALVES = [(0, 512), (512, 490)]               # matmul N<=512 splits of TC

F32 = mybir.dt.float32
F32R = mybir.dt.float32r
BF16 = mybir.dt.bfloat16
AF = mybir.ActivationFunctionType

_NC_CACHE = {}


def _budget(inst) -> int:
    return 1


def fix_sync_waits(nc):
    """Split per-instruction sync waits: this walrus build allows only ONE
    sync wait per instruction; hoist extras onto NoOps just before, on the
    same engine queue (engine streams execute in block order)."""
    counter = 0
    for f in nc.m.functions:
        for bb in f.blocks:
            new_insts = []
            changed = False
            for inst in bb.instructions:
                si = inst.sync_info
                waits = list(si.on_wait) if si is not None and si.on_wait else []
                budget = _budget(inst)
                if len(waits) > budget:
                    keep = waits[len(waits) - budget:]
                    excess = waits[: len(waits) - budget]
                    for wt in excess:
                        counter += 1
                        nop = mybir.InstNoOp(name=f"I-waitfix-{counter}")
                        nop.engine = inst.engine
                        nop.sync_info = mybir.SyncInfo(on_wait=[wt], on_update=[])
                        new_insts.append(nop)
                    inst.sync_info = mybir.SyncInfo(
                        on_wait=keep,
                        on_update=list(si.on_update) if si.on_update else [],
                    )
                    changed = True
                new_insts.append(inst)
            if changed:
                bb.instructions = new_insts
    return nc


def build_nc():
    nc = bass.Bass()

    frames = nc.declare_dram_parameter("frames", [2, NFFT, TC], F32, isOutput=False)
    dftc = nc.declare_dram_parameter("dftc", [NFT, 128, NCH, 128], F32, isOutput=False)
    dfts = nc.declare_dram_parameter("dfts", [NFT, 128, NCH, 128], F32, isOutput=False)
    wwr = nc.declare_dram_parameter("wwr", [NDT, 128, NFT, 128], BF16, isOutput=False)
    wwi = nc.declare_dram_parameter("wwi", [NDT, 128, NFT, 128], BF16, isOutput=False)
    whr = nc.declare_dram_parameter("whr", [NDT, 128, NFT, 128], BF16, isOutput=False)
    whl = nc.declare_dram_parameter("whl", [NDT, 128, NFT, 128], BF16, isOutput=False)
    melw = nc.declare_dram_parameter("melw", [128, NFT, NM], BF16, isOutput=False)
    melwf = nc.declare_dram_parameter("melwf", [128, NFT, NM], F32, isOutput=False)
    sTw = nc.declare_dram_parameter("sTw", [NDT, 128, NM], BF16, isOutput=False)
    rcnt = nc.declare_dram_parameter("rcnt", [NM, 1], F32, isOutput=False)
    out = nc.declare_dram_parameter("out", [6, NM, T], F32, isOutput=True)

    with tile.TileContext(nc) as tc, ExitStack() as ctx:
        # resident pow/csd arrays [128, 9, 1001] bf16
        resid = ctx.enter_context(tc.tile_pool(name="resid", bufs=1))
        powL = resid.tile([128, NFT, TC], BF16, tag="powL")
        powR = resid.tile([128, NFT, TC], BF16, tag="powR")
        csdR = resid.tile([128, NFT, TC], BF16, tag="csdR")
        csdI = resid.tile([128, NFT, TC], BF16, tag="csdI")
        # mel/channel tiles [64, 1001] f32 + consts
        mels = ctx.enter_context(tc.tile_pool(name="mels", bufs=1))
        melL_s = mels.tile([NM, TC], F32, tag="melL")
        melR_s = mels.tile([NM, TC], F32, tag="melR")
        melr_s = mels.tile([NM, TC], F32, tag="melr")
        meli_s = mels.tile([NM, TC], F32, tag="meli")
        eb128 = mels.tile([128, 1], F32, tag="eb128")
        rcnt_s = mels.tile([NM, 1], F32, tag="rcnt")
        nc.vector.memset(eb128, EPS)
        nc.sync.dma_start(out=rcnt_s, in_=rcnt[:, :])
        eb64 = eb128[:NM, :]
        # f32 csd staging in DRAM (for high-precision mel_csd path: ch3/ch4)
        dpool = ctx.enter_context(tc.tile_pool(name="csdd", bufs=1, space="DRAM"))
        csdr32 = dpool.tile([128, NFT, TC], F32, tag="csdr32")
        csdi32 = dpool.tile([128, NFT, TC], F32, tag="csdi32")

        # ---------------- Phase A: STFT + pow/csd ----------------
        with tc.tile_pool(name="frames", bufs=1) as fpool, \
             tc.tile_pool(name="dftw", bufs=2) as wpool, \
             tc.tile_pool(name="patmp", bufs=2) as tpool, \
             tc.tile_pool(name="psumA", bufs=2, space="PSUM") as psA:
            for (t0, tcnt) in T_PASSES:
                ftiles = {}
                for ch in range(2):
                    for c in range(NCH):
                        ft = fpool.tile([128, tcnt], F32R, tag=f"f{ch}_{c}")
                        nc.sync.dma_start(
                            out=ft,
                            in_=frames[ch, 128 * c:128 * (c + 1),
                                       t0:t0 + tcnt].bitcast(F32R))
                        ftiles[(ch, c)] = ft
                for fi in range(NFT):
                    wc = wpool.tile([128, NCH, 128], F32R, tag="wc")
                    nc.sync.dma_start(out=wc, in_=dftc[fi].bitcast(F32R))
                    ws = wpool.tile([128, NCH, 128], F32R, tag="ws")
                    nc.sync.dma_start(out=ws, in_=dfts[fi].bitcast(F32R))
                    plr = psA.tile([128, tcnt], F32, tag="plr")
                    pli = psA.tile([128, tcnt], F32, tag="pli")
                    prr = psA.tile([128, tcnt], F32, tag="prr")
                    pri = psA.tile([128, tcnt], F32, tag="pri")
                    for c in range(NCH):
                        st, sp = (c == 0), (c == NCH - 1)
                        nc.tensor.matmul(plr, wc[:, c, :], ftiles[(0, c)],
                                         start=st, stop=sp, skip_group_check=True)
                        nc.tensor.matmul(prr, wc[:, c, :], ftiles[(1, c)],
                                         start=st, stop=sp, skip_group_check=True)
                        nc.tensor.matmul(pli, ws[:, c, :], ftiles[(0, c)],
                                         start=st, stop=sp, skip_group_check=True)
                        nc.tensor.matmul(pri, ws[:, c, :], ftiles[(1, c)],
                                         start=st, stop=sp, skip_group_check=True)
                    # derive pow/csd for this (pass, fi)
                    tsl = slice(t0, t0 + tcnt)
                    t1 = tpool.tile([128, tcnt], F32, tag="t1")
                    t2 = tpool.tile([128, tcnt], F32, tag="t2")
                    t3 = tpool.tile([128, tcnt], F32, tag="t3")
                    t4 = tpool.tile([128, tcnt], F32, tag="t4")
                    a_r = tpool.tile([128, tcnt], F32, tag="a_r")
                    a_i = tpool.tile([128, tcnt], F32, tag="a_i")
                    nc.scalar.copy(a_r, plr)
                    nc.scalar.copy(a_i, pli)
                    nc.scalar.activation(t1, plr, AF.Square)
                    nc.scalar.activation(t2, pli, AF.Square)
                    nc.scalar.activation(t3, prr, AF.Square)
                    nc.scalar.activation(t4, pri, AF.Square)
                    nc.vector.tensor_add(powL[:, fi, tsl], t1, t2)
                    nc.vector.tensor_add(powR[:, fi, tsl], t3, t4)
                    m1 = tpool.tile([128, tcnt], F32, tag="m1")
                    m2 = tpool.tile([128, tcnt], F32, tag="m2")
                    nc.vector.tensor_mul(m1, a_r, prr)
                    nc.vector.tensor_mul(m2, a_i, pri)
                    csdf = tpool.tile([128, tcnt], F32, tag="csdf")
                    nc.vector.tensor_add(csdf, m1, m2)
                    nc.scalar.copy(csdR[:, fi, tsl], csdf)
                    nc.sync.dma_start(out=csdr32[:, fi, tsl], in_=csdf)
                    m3 = tpool.tile([128, tcnt], F32, tag="m3")
                    m4 = tpool.tile([128, tcnt], F32, tag="m4")
                    nc.vector.tensor_mul(m3, a_i, prr)
                    nc.vector.tensor_mul(m4, a_r, pri)
                    csif = tpool.tile([128, tcnt], F32, tag="csif")
                    nc.vector.tensor_sub(csif, m3, m4)
                    nc.scalar.copy(csdI[:, fi, tsl], csif)
                    nc.sync.dma_start(out=csdi32[:, fi, tsl], in_=csif)

        # ---------------- Mel phase ----------------
        with tc.tile_pool(name="melwp", bufs=1) as mwp, \
             tc.tile_pool(name="melrhs", bufs=4) as mrp, \
             tc.tile_pool(name="psumM", bufs=1, space="PSUM") as psM:
            melw_s = mwp.tile([128, NFT, NM], BF16, tag="melw")
            nc.sync.dma_start(out=melw_s, in_=melw[:, :, :])
            melwf_s = mwp.tile([128, NFT, NM], F32, tag="melwf")
            nc.sync.dma_start(out=melwf_s, in_=melwf[:, :, :])
            # pow -> mel in bf16 (log-scale outputs are tolerant)
            for (name, rhs, dst) in (("mL", powL, melL_s), ("mR", powR, melR_s)):
                pm = psM.tile([NM, TC], F32, tag=f"pm_{name}")
                for fi in range(NFT):
                    for (h0, hc) in T_HALVES:
                        nc.tensor.matmul(
                            pm[:, h0:h0 + hc], melw_s[:, fi, :],
                            rhs[:, fi, h0:h0 + hc],
                            start=(fi == 0), stop=(fi == NFT - 1),
                            skip_group_check=True)
                nc.scalar.copy(dst, pm)
            # csd -> mel in full fp32 (ch3/ch4 are ill-conditioned where
            # mel_csd cancels; bf16 there saturates the direction error)
            for (name, src, dst) in (("mr", csdr32, melr_s), ("mi", csdi32, meli_s)):
                pm = psM.tile([NM, TC], F32, tag=f"pm_{name}")
                for fi in range(NFT):
                    for (h0, hc) in T_HALVES:
                        rt = mrp.tile([128, hc], F32, tag=f"rt{hc}")
                        nc.sync.dma_start(out=rt, in_=src[:, fi, h0:h0 + hc])
                        nc.tensor.matmul(
                            pm[:, h0:h0 + hc], melwf_s[:, fi, :], rt,
                            start=(fi == 0), stop=(fi == NFT - 1),
                            skip_group_check=True)
                nc.scalar.copy(dst, pm)

        # ---------------- Phase B: corr + scatter ----------------
        with tc.tile_pool(name="wB", bufs=2) as wBp, \
             tc.tile_pool(name="pbt", bufs=2) as pbt, \
             tc.tile_pool(name="psumB", bufs=1, space="PSUM") as psB:
            ch5p = psB.tile([NM, TC], F32, tag="ch5")
            for dj in range(NDT):
                whr_t = wBp.tile([128, NFT, 128], BF16, tag="whr")
                nc.sync.dma_start(out=whr_t, in_=whr[dj])
                whl_t = wBp.tile([128, NFT, 128], BF16, tag="whl")
                nc.sync.dma_start(out=whl_t, in_=whl[dj])
                wwr_t = wBp.tile([128, NFT, 128], BF16, tag="wwr")
                nc.sync.dma_start(out=wwr_t, in_=wwr[dj])
                wwi_t = wBp.tile([128, NFT, 128], BF16, tag="wwi")
                nc.sync.dma_start(out=wwi_t, in_=wwi[dj])
                sT_t = wBp.tile([128, NM], BF16, tag="sT")
                nc.sync.dma_start(out=sT_t, in_=sTw[dj])

                n1 = psB.tile([128, TC], F32, tag="n1")
                n2 = psB.tile([128, TC], F32, tag="n2")
                cu = psB.tile([128, TC], F32, tag="cu")
                for fi in range(NFT):
                    st, sp = (fi == 0), (fi == NFT - 1)
                    for (h0, hc) in T_HALVES:
                        hs = slice(h0, h0 + hc)
                        nc.tensor.matmul(n1[:, hs], whr_t[:, fi, :],
                                         powL[:, fi, hs], start=st, stop=sp,
                                         skip_group_check=True)
                        nc.tensor.matmul(n2[:, hs], whl_t[:, fi, :],
                                         powR[:, fi, hs], start=st, stop=sp,
                                         skip_group_check=True)
                        nc.tensor.matmul(cu[:, hs], wwr_t[:, fi, :],
                                         csdR[:, fi, hs], start=st, stop=False,
                                         skip_group_check=True)
                        nc.tensor.matmul(cu[:, hs], wwi_t[:, fi, :],
                                         csdI[:, fi, hs], start=False, stop=sp,
                                         skip_group_check=True)
                cn1 = pbt.tile([128, TC], F32, tag="cn1")
                nc.scalar.copy(cn1, n1)
                den = pbt.tile([128, TC], F32, tag="den")
                nc.vector.tensor_mul(den, cn1, n2)
                sq = pbt.tile([128, TC], F32, tag="sq")
                nc.scalar.activation(sq, den, AF.Sqrt, bias=eb128)
                rden = pbt.tile([128, TC], F32, tag="rden")
                nc.vector.reciprocal(rden, sq)
                corr = pbt.tile([128, TC], BF16, tag="corr")
                nc.vector.tensor_mul(corr, cu, rden)
                for (h0, hc) in T_HALVES:
                    hs = slice(h0, h0 + hc)
                    nc.tensor.matmul(ch5p[:, hs], sT_t, corr[:, hs],
                                     start=(dj == 0), stop=(dj == NDT - 1),
                                     skip_group_check=True)

            # ---------------- Phase C: finalize ----------------
            with tc.tile_pool(name="fint", bufs=1) as fin:
                # ch0 / ch1
                u0 = fin.tile([NM, TC], F32, tag="u0")
                nc.scalar.activation(u0, melL_s, AF.Ln, bias=eb64)
                c0 = fin.tile([NM, TC], F32, tag="c0")
                nc.scalar.mul(c0, u0, K_LOG)
                nc.sync.dma_start(out=out[0], in_=c0[:, :T])
                u1 = fin.tile([NM, TC], F32, tag="u1")
                nc.scalar.activation(u1, melR_s, AF.Ln, bias=eb64)
                c1 = fin.tile([NM, TC], F32, tag="c1")
                nc.scalar.mul(c1, u1, K_LOG)
                nc.sync.dma_start(out=out[1], in_=c1[:, :T])
                # ch2 = K*ln(melL/(melR+eps) + eps)
                v2 = fin.tile([NM, TC], F32, tag="v2")
                nc.scalar.activation(v2, melR_s, AF.Identity, bias=eb64)
                r2 = fin.tile([NM, TC], F32, tag="r2")
                nc.vector.reciprocal(r2, v2)
                w2 = fin.tile([NM, TC], F32, tag="w2")
                nc.vector.tensor_mul(w2, melL_s, r2)
                u2 = fin.tile([NM, TC], F32, tag="u2")
                nc.scalar.activation(u2, w2, AF.Ln, bias=eb64)
                c2 = fin.tile([NM, TC], F32, tag="c2")
                nc.scalar.mul(c2, u2, K_LOG)
                nc.sync.dma_start(out=out[2], in_=c2[:, :T])
                # ch3/ch4
                s1 = fin.tile([NM, TC], F32, tag="s1")
                nc.scalar.activation(s1, melr_s, AF.Square)
                s2 = fin.tile([NM, TC], F32, tag="s2")
                nc.scalar.activation(s2, meli_s, AF.Square)
                ss = fin.tile([NM, TC], F32, tag="ss")
                nc.vector.tensor_add(ss, s1, s2)
                nrm = fin.tile([NM, TC], F32, tag="nrm")
                nc.scalar.activation(nrm, ss, AF.Sqrt, bias=eb64)
                rn = fin.tile([NM, TC], F32, tag="rn")
                nc.vector.reciprocal(rn, nrm)
                c3 = fin.tile([NM, TC], F32, tag="c3")
                nc.vector.tensor_mul(c3, meli_s, rn)
                nc.sync.dma_start(out=out[3], in_=c3[:, :T])
                c4 = fin.tile([NM, TC], F32, tag="c4")
                nc.vector.tensor_mul(c4, melr_s, rn)
                nc.sync.dma_start(out=out[4], in_=c4[:, :T])
                # ch5 = ch5p * rcnt (per-partition scalar)
                c5 = fin.tile([NM, TC], F32, tag="c5")
                nc.vector.tensor_scalar_mul(c5, ch5p, rcnt_s)
                nc.sync.dma_start(out=out[5], in_=c5[:, :T])

    fix_sync_waits(nc)
    return nc


def _host_prep(inputs):
    wav = np.asarray(inputs["waveform"], dtype=np.float32)          # [8,2,NS]
    W_real = np.asarray(inputs["W_real"], dtype=np.float32)         # [ND,F]
    W_imag = np.asarray(inputs["W_imag"], dtype=np.float32)
    norm_hr = np.asarray(inputs["norm_hr_sq"], dtype=np.float32)
    norm_hl = np.asarray(inputs["norm_hl_sq"], dtype=np.float32)
    az = np.asarray(inputs["az_bin_idx"]).astype(np.int64)          # [ND]
    win = np.asarray(inputs["window"], dtype=np.float32)            # [NFFT]
    mel_fb = np.asarray(inputs["mel_fb"], dtype=np.float32)         # [NM,F]

    xpad = np.pad(wav, ((0, 0), (0, 0), (NFFT // 2, NFFT // 2 + HOP)))  # [8,2,PADNS]
    # host framing: [2, NFFT, TC] per batch, so device frame chunks are
    # partition-contiguous (the strided on-device gather exploded into
    # per-element 4B DMA descriptors)
    sw = np.lib.stride_tricks.sliding_window_view(xpad, NFFT, axis=2)[:, :, ::HOP]
    # sw: [8, 2, TC, NFFT] view -> transpose to [8, 2, NFFT, TC] contiguous
    frames_np = np.ascontiguousarray(np.swapaxes(sw, 2, 3))

    n = np.arange(NFFT, dtype=np.float64)[:, None]
    k = np.arange(F, dtype=np.float64)[None, :]
    ang = 2.0 * np.pi * n * k / NFFT
    dftc = np.zeros((NFFT, FP), dtype=np.float32)
    dfts = np.zeros((NFFT, FP), dtype=np.float32)
    dftc[:, :F] = (np.cos(ang) * win[:, None]).astype(np.float32)
    dfts[:, :F] = (-np.sin(ang) * win[:, None]).astype(np.float32)

    def tile_dft(a):  # [NFFT, FP] -> [NFT, 128, NCH, 128]
        return np.ascontiguousarray(
            a.reshape(NCH, 128, NFT, 128).transpose(2, 1, 0, 3))

    def tile_w(mat):  # [ND, F] -> lhsT tiled [NDT, 128, NFT, 128] bf16
        t = np.zeros((FP, ND), dtype=np.float32)
        t[:F] = mat.T
        return np.ascontiguousarray(
            t.reshape(NFT, 128, NDT, 128).transpose(2, 1, 0, 3)
        ).astype(ml_dtypes.bfloat16)

    wwr = tile_w(W_real)
    wwi = tile_w(-W_imag)
    whr = tile_w(norm_hr)
    whl = tile_w(norm_hl)

    melT = np.zeros((FP, NM), dtype=np.float32)
    melT[:F] = mel_fb.T
    melwf = np.ascontiguousarray(melT.reshape(NFT, 128, NM).transpose(1, 0, 2))
    melw = melwf.astype(ml_dtypes.bfloat16)

    cnt = np.bincount(az, minlength=NM).astype(np.float32)
    S01 = (az[:, None] == np.arange(NM)[None, :]).astype(np.float32)  # [ND,NM]
    sTw = np.ascontiguousarray(S01.reshape(NDT, 128, NM)).astype(ml_dtypes.bfloat16)
    rcnt = (1.0 / (cnt + EPS)).astype(np.float32).reshape(NM, 1)

    shared = {
        "dftc": tile_dft(dftc), "dfts": tile_dft(dfts),
        "wwr": wwr, "wwi": wwi, "whr": whr, "whl": whl,
        "melw": melw, "melwf": melwf, "sTw": sTw, "rcnt": rcnt,
    }
    in_maps = []
    for b in range(B):
        m = dict(shared)
        m["frames"] = frames_np[b]
        in_maps.append(m)
    return in_maps


def kernel(**inputs) -> np.ndarray:
    if "nc" not in _NC_CACHE:
        _NC_CACHE["nc"] = build_nc()
    nc = _NC_CACHE["nc"]
    in_maps = _host_prep(inputs)
    res = run_bass_kernel_spmd(nc, in_maps, core_ids=list(range(B)))
    out = np.stack([np.asarray(res.results[i]["out"]) for i in range(B)])
    return out.astype(np.float32)


# revision 22
# speedup vs baseline: 3.9501x; 1.0468x over previous
"""Trainium2 Bass kernel for nn_AudioPreprocessor (binaural STFT features).

Contract: kernel(**inputs) takes the FULL unsharded inputs (numpy) and
returns the full [8, 6, 64, 1001] float32 output. Internally: data-parallel
over batch across 8 NeuronCores (one batch per core, no collectives).

Pipeline per core (batch b):
  A) STFT of L/R channels as DFT-matmuls (f32r), accumulate in PSUM over
     16 contraction chunks; derive pow_L, pow_R, csd_r, csd_i into SBUF
     (bf16, resident [128, 9, 1001] per array, freq-padded 1025->1152).
  M) mel_fb matmuls -> mel_L/R, mel_csd_r/i [64, 1001].
  B) per direction tile (18 x 128 dirs): n1/n2/corr_un matmuls (bf16
     weights streamed), normalize corr = corr_un*rsqrt(n1*n2+1e-8),
     scatter into azimuth bins via 0/1 matmul accumulated in PSUM.
  C) finalize ch0..ch5, DMA out [6, 64, 1001].
"""
import sys
import os

_RL = "/opt/trn_rl_repo"
if _RL not in sys.path:
    sys.path.insert(0, _RL)

import numpy as np
import ml_dtypes
from contextlib import ExitStack

import concourse.bass as bass
import concourse.mybir as mybir
import concourse.tile as tile
from concourse.bass_utils import run_bass_kernel_spmd

# ---------------- problem constants (hardcoded) ----------------
B = 8
NS = 960000
NFFT = 2048
HOP = 960
F = 1025
FP = 1152            # padded freq dim: 9 * 128
NFT = FP // 128      # 9 freq tiles
NCH = 16             # 2048 / 128 contraction chunks
NM = 64
ND = 2304
NDT = ND // 128      # 18 direction tiles
T = 1001
TC = 1002           # computed columns (f32r matmul needs even N); last col discarded
EPS = 1e-8
PADNS = NS + NFFT + HOP   # 963008 (one extra hop so we can compute T_C=1002 columns)
K_LOG = 10.0 / np.log(10.0)

T_PASSES = [(0, 334), (334, 334), (668, 334)]   # phase A (N>=256, even, for f32r)
T_HALVES = [(0, 512), (512, 490)]               # matmul N<=512 splits of TC

F32 = mybir.dt.float32
F32R = mybir.dt.float32r
BF16 = mybir.dt.bfloat16
AF = mybir.ActivationFunctionType

_NC_CACHE = {}


def _budget(inst) -> int:
    return 1


def fix_sync_waits(nc):
    """Split per-instruction sync waits: this walrus build allows only ONE
    sync wait per instruction; hoist extras onto NoOps just before, on the
    same engine queue (engine streams execute in block order)."""
    counter = 0
    for f in nc.m.functions:
        for bb in f.blocks:
            new_insts = []
            changed = False
            for inst in bb.instructions:
                si = inst.sync_info
                waits = list(si.on_wait) if si is not None and si.on_wait else []
                budget = _budget(inst)
                if len(waits) > budget:
                    keep = waits[len(waits) - budget:]
                    excess = waits[: len(waits) - budget]
                    for wt in excess:
                        counter += 1
                        nop = mybir.InstNoOp(name=f"I-waitfix-{counter}")
                        nop.engine = inst.engine
                        nop.sync_info = mybir.SyncInfo(on_wait=[wt], on_update=[])
                        new_insts.append(nop)
                    inst.sync_info = mybir.SyncInfo(
                        on_wait=keep,
                        on_update=list(si.on_update) if si.on_update else [],
                    )
                    changed = True
                new_insts.append(inst)
            if changed:
                bb.instructions = new_insts
    return nc


def build_nc():
    nc = bass.Bass()

    frames = nc.declare_dram_parameter("frames", [2, NFFT, TC], F32, isOutput=False)
    dftc = nc.declare_dram_parameter("dftc", [NFT, 128, NCH, 128], F32, isOutput=False)
    dfts = nc.declare_dram_parameter("dfts", [NFT, 128, NCH, 128], F32, isOutput=False)
    wwr = nc.declare_dram_parameter("wwr", [NDT, 128, NFT, 128], BF16, isOutput=False)
    wwi = nc.declare_dram_parameter("wwi", [NDT, 128, NFT, 128], BF16, isOutput=False)
    whr = nc.declare_dram_parameter("whr", [NDT, 128, NFT, 128], BF16, isOutput=False)
    whl = nc.declare_dram_parameter("whl", [NDT, 128, NFT, 128], BF16, isOutput=False)
    melw = nc.declare_dram_parameter("melw", [128, NFT, NM], BF16, isOutput=False)
    melwf = nc.declare_dram_parameter("melwf", [128, NFT, NM], F32, isOutput=False)
    sTw = nc.declare_dram_parameter("sTw", [NDT, 128, NM], BF16, isOutput=False)
    rcnt = nc.declare_dram_parameter("rcnt", [NM, 1], F32, isOutput=False)
    out = nc.declare_dram_parameter("out", [6, NM, T], F32, isOutput=True)

    with tile.TileContext(nc) as tc, ExitStack() as ctx:
        # resident pow/csd arrays [128, 9, 1001] bf16
        resid = ctx.enter_context(tc.tile_pool(name="resid", bufs=1))
        powL = resid.tile([128, NFT, TC], BF16, tag="powL")
        powR = resid.tile([128, NFT, TC], BF16, tag="powR")
        csdR = resid.tile([128, NFT, TC], BF16, tag="csdR")
        csdI = resid.tile([128, NFT, TC], BF16, tag="csdI")
        # mel/channel tiles [64, 1001] f32 + consts
        mels = ctx.enter_context(tc.tile_pool(name="mels", bufs=1))
        melL_s = mels.tile([NM, TC], F32, tag="melL")
        melR_s = mels.tile([NM, TC], F32, tag="melR")
        melr_s = mels.tile([NM, TC], F32, tag="melr")
        meli_s = mels.tile([NM, TC], F32, tag="meli")
        eb128 = mels.tile([128, 1], F32, tag="eb128")
        rcnt_s = mels.tile([NM, 1], F32, tag="rcnt")
        nc.vector.memset(eb128, EPS)
        nc.sync.dma_start(out=rcnt_s, in_=rcnt[:, :])
        eb64 = eb128[:NM, :]
        # f32 csd staging in DRAM (for high-precision mel_csd path: ch3/ch4)
        dpool = ctx.enter_context(tc.tile_pool(name="csdd", bufs=1, space="DRAM"))
        csdr32 = dpool.tile([128, NFT, TC], F32, tag="csdr32")
        csdi32 = dpool.tile([128, NFT, TC], F32, tag="csdi32")

        # ---------------- Phase A: STFT + pow/csd ----------------
        with tc.tile_pool(name="frames", bufs=1) as fpool, \
             tc.tile_pool(name="dftw", bufs=2) as wpool, \
             tc.tile_pool(name="patmp", bufs=2) as tpool, \
             tc.tile_pool(name="psumA", bufs=2, space="PSUM") as psA:
            for (t0, tcnt) in T_PASSES:
                ftiles = {}
                for ch in range(2):
                    for c in range(NCH):
                        ft = fpool.tile([128, tcnt], F32R, tag=f"f{ch}_{c}")
                        eng = nc.sync if (c % 2 == 0) else nc.scalar
                        eng.dma_start(
                            out=ft,
                            in_=frames[ch, 128 * c:128 * (c + 1),
                                       t0:t0 + tcnt].bitcast(F32R))
                        ftiles[(ch, c)] = ft
                for fi in range(NFT):
                    wc = wpool.tile([128, NCH, 128], F32R, tag="wc")
                    nc.sync.dma_start(out=wc, in_=dftc[fi].bitcast(F32R))
                    ws = wpool.tile([128, NCH, 128], F32R, tag="ws")
                    nc.scalar.dma_start(out=ws, in_=dfts[fi].bitcast(F32R))
                    plr = psA.tile([128, tcnt], F32, tag="plr")
                    pli = psA.tile([128, tcnt], F32, tag="pli")
                    prr = psA.tile([128, tcnt], F32, tag="prr")
                    pri = psA.tile([128, tcnt], F32, tag="pri")
                    for c in range(NCH):
                        st, sp = (c == 0), (c == NCH - 1)
                        nc.tensor.matmul(plr, wc[:, c, :], ftiles[(0, c)],
                                         start=st, stop=sp, skip_group_check=True)
                        nc.tensor.matmul(prr, wc[:, c, :], ftiles[(1, c)],
                                         start=st, stop=sp, skip_group_check=True)
                        nc.tensor.matmul(pli, ws[:, c, :], ftiles[(0, c)],
                                         start=st, stop=sp, skip_group_check=True)
                        nc.tensor.matmul(pri, ws[:, c, :], ftiles[(1, c)],
                                         start=st, stop=sp, skip_group_check=True)
                    # derive pow/csd for this (pass, fi)
                    tsl = slice(t0, t0 + tcnt)
                    t1 = tpool.tile([128, tcnt], F32, tag="t1")
                    t2 = tpool.tile([128, tcnt], F32, tag="t2")
                    t3 = tpool.tile([128, tcnt], F32, tag="t3")
                    t4 = tpool.tile([128, tcnt], F32, tag="t4")
                    a_r = tpool.tile([128, tcnt], F32, tag="a_r")
                    a_i = tpool.tile([128, tcnt], F32, tag="a_i")
                    nc.scalar.copy(a_r, plr)
                    nc.scalar.copy(a_i, pli)
                    nc.scalar.activation(t1, plr, AF.Square)
                    nc.scalar.activation(t2, pli, AF.Square)
                    nc.scalar.activation(t3, prr, AF.Square)
                    nc.scalar.activation(t4, pri, AF.Square)
                    nc.vector.tensor_add(powL[:, fi, tsl], t1, t2)
                    nc.vector.tensor_add(powR[:, fi, tsl], t3, t4)
                    m1 = tpool.tile([128, tcnt], F32, tag="m1")
                    m2 = tpool.tile([128, tcnt], F32, tag="m2")
                    nc.vector.tensor_mul(m1, a_r, prr)
                    nc.vector.tensor_mul(m2, a_i, pri)
                    csdf = tpool.tile([128, tcnt], F32, tag="csdf")
                    nc.vector.tensor_add(csdf, m1, m2)
                    nc.scalar.copy(csdR[:, fi, tsl], csdf)
                    nc.sync.dma_start(out=csdr32[:, fi, tsl], in_=csdf)
                    m3 = tpool.tile([128, tcnt], F32, tag="m3")
                    m4 = tpool.tile([128, tcnt], F32, tag="m4")
                    nc.vector.tensor_mul(m3, a_i, prr)
                    nc.vector.tensor_mul(m4, a_r, pri)
                    csif = tpool.tile([128, tcnt], F32, tag="csif")
                    nc.vector.tensor_sub(csif, m3, m4)
                    nc.scalar.copy(csdI[:, fi, tsl], csif)
                    nc.sync.dma_start(out=csdi32[:, fi, tsl], in_=csif)

        # ---------------- Mel phase ----------------
        with tc.tile_pool(name="melwp", bufs=1) as mwp, \
             tc.tile_pool(name="melrhs", bufs=4) as mrp, \
             tc.tile_pool(name="psumM", bufs=1, space="PSUM") as psM:
            melw_s = mwp.tile([128, NFT, NM], BF16, tag="melw")
            nc.sync.dma_start(out=melw_s, in_=melw[:, :, :])
            melwf_s = mwp.tile([128, NFT, NM], F32, tag="melwf")
            nc.sync.dma_start(out=melwf_s, in_=melwf[:, :, :])
            # pow -> mel in bf16 (log-scale outputs are tolerant)
            for (name, rhs, dst) in (("mL", powL, melL_s), ("mR", powR, melR_s)):
                pm = psM.tile([NM, TC], F32, tag=f"pm_{name}")
                for fi in range(NFT):
                    for (h0, hc) in T_HALVES:
                        nc.tensor.matmul(
                            pm[:, h0:h0 + hc], melw_s[:, fi, :],
                            rhs[:, fi, h0:h0 + hc],
                            start=(fi == 0), stop=(fi == NFT - 1),
                            skip_group_check=True)
                nc.scalar.copy(dst, pm)
            # csd -> mel in full fp32 (ch3/ch4 are ill-conditioned where
            # mel_csd cancels; bf16 there saturates the direction error)
            for (name, src, dst) in (("mr", csdr32, melr_s), ("mi", csdi32, meli_s)):
                pm = psM.tile([NM, TC], F32, tag=f"pm_{name}")
                for fi in range(NFT):
                    for (h0, hc) in T_HALVES:
                        rt = mrp.tile([128, hc], F32, tag=f"rt{hc}")
                        nc.sync.dma_start(out=rt, in_=src[:, fi, h0:h0 + hc])
                        nc.tensor.matmul(
                            pm[:, h0:h0 + hc], melwf_s[:, fi, :], rt,
                            start=(fi == 0), stop=(fi == NFT - 1),
                            skip_group_check=True)
                nc.scalar.copy(dst, pm)

        # ---------------- Phase B: corr + scatter ----------------
        with tc.tile_pool(name="wB", bufs=2) as wBp, \
             tc.tile_pool(name="pbt", bufs=2) as pbt, \
             tc.tile_pool(name="psumB", bufs=2, space="PSUM") as psB, \
             tc.tile_pool(name="psumB5", bufs=1, space="PSUM") as psB5:
            ch5p = psB5.tile([NM, TC], F32, tag="ch5")
            for dj in range(NDT):
                whr_t = wBp.tile([128, NFT, 128], BF16, tag="whr")
                nc.sync.dma_start(out=whr_t, in_=whr[dj])
                whl_t = wBp.tile([128, NFT, 128], BF16, tag="whl")
                nc.scalar.dma_start(out=whl_t, in_=whl[dj])
                wwr_t = wBp.tile([128, NFT, 128], BF16, tag="wwr")
                nc.sync.dma_start(out=wwr_t, in_=wwr[dj])
                wwi_t = wBp.tile([128, NFT, 128], BF16, tag="wwi")
                nc.scalar.dma_start(out=wwi_t, in_=wwi[dj])
                sT_t = wBp.tile([128, NM], BF16, tag="sT")
                nc.sync.dma_start(out=sT_t, in_=sTw[dj])

                for (h0, hc) in T_HALVES:
                    hs = slice(h0, h0 + hc)
                    n1 = psB.tile([128, hc], F32, tag="n1")
                    n2 = psB.tile([128, hc], F32, tag="n2")
                    cu = psB.tile([128, hc], F32, tag="cu")
                    for fi in range(NFT):
                        st, sp = (fi == 0), (fi == NFT - 1)
                        nc.tensor.matmul(n1, whr_t[:, fi, :],
                                         powL[:, fi, hs], start=st, stop=sp,
                                         skip_group_check=True)
                        nc.tensor.matmul(n2, whl_t[:, fi, :],
                                         powR[:, fi, hs], start=st, stop=sp,
                                         skip_group_check=True)
                        nc.tensor.matmul(cu, wwr_t[:, fi, :],
                                         csdR[:, fi, hs], start=st, stop=False,
                                         skip_group_check=True)
                        nc.tensor.matmul(cu, wwi_t[:, fi, :],
                                         csdI[:, fi, hs], start=False, stop=sp,
                                         skip_group_check=True)
                    cn1 = pbt.tile([128, hc], F32, tag="cn1")
                    nc.scalar.copy(cn1, n1)
                    den = pbt.tile([128, hc], F32, tag="den")
                    nc.vector.tensor_mul(den, cn1, n2)
                    sq = pbt.tile([128, hc], F32, tag="sq")
                    nc.scalar.activation(sq, den, AF.Sqrt, bias=eb128)
                    rden = pbt.tile([128, hc], F32, tag="rden")
                    nc.vector.reciprocal(rden, sq)
                    corr = pbt.tile([128, hc], BF16, tag="corr")
                    nc.vector.tensor_mul(corr, cu, rden)
                    nc.tensor.matmul(ch5p[:, hs], sT_t, corr,
                                     start=(dj == 0), stop=(dj == NDT - 1),
                                     skip_group_check=True)

            # ---------------- Phase C: finalize ----------------
            with tc.tile_pool(name="fint", bufs=1) as fin:
                # ch0 / ch1
                u0 = fin.tile([NM, TC], F32, tag="u0")
                nc.scalar.activation(u0, melL_s, AF.Ln, bias=eb64)
                c0 = fin.tile([NM, TC], F32, tag="c0")
                nc.scalar.mul(c0, u0, K_LOG)
                nc.sync.dma_start(out=out[0], in_=c0[:, :T])
                u1 = fin.tile([NM, TC], F32, tag="u1")
                nc.scalar.activation(u1, melR_s, AF.Ln, bias=eb64)
                c1 = fin.tile([NM, TC], F32, tag="c1")
                nc.scalar.mul(c1, u1, K_LOG)
                nc.sync.dma_start(out=out[1], in_=c1[:, :T])
                # ch2 = K*ln(melL/(melR+eps) + eps)
                v2 = fin.tile([NM, TC], F32, tag="v2")
                nc.scalar.activation(v2, melR_s, AF.Identity, bias=eb64)
                r2 = fin.tile([NM, TC], F32, tag="r2")
                nc.vector.reciprocal(r2, v2)
                w2 = fin.tile([NM, TC], F32, tag="w2")
                nc.vector.tensor_mul(w2, melL_s, r2)
                u2 = fin.tile([NM, TC], F32, tag="u2")
                nc.scalar.activation(u2, w2, AF.Ln, bias=eb64)
                c2 = fin.tile([NM, TC], F32, tag="c2")
                nc.scalar.mul(c2, u2, K_LOG)
                nc.sync.dma_start(out=out[2], in_=c2[:, :T])
                # ch3/ch4
                s1 = fin.tile([NM, TC], F32, tag="s1")
                nc.scalar.activation(s1, melr_s, AF.Square)
                s2 = fin.tile([NM, TC], F32, tag="s2")
                nc.scalar.activation(s2, meli_s, AF.Square)
                ss = fin.tile([NM, TC], F32, tag="ss")
                nc.vector.tensor_add(ss, s1, s2)
                nrm = fin.tile([NM, TC], F32, tag="nrm")
                nc.scalar.activation(nrm, ss, AF.Sqrt, bias=eb64)
                rn = fin.tile([NM, TC], F32, tag="rn")
                nc.vector.reciprocal(rn, nrm)
                c3 = fin.tile([NM, TC], F32, tag="c3")
                nc.vector.tensor_mul(c3, meli_s, rn)
                nc.sync.dma_start(out=out[3], in_=c3[:, :T])
                c4 = fin.tile([NM, TC], F32, tag="c4")
                nc.vector.tensor_mul(c4, melr_s, rn)
                nc.sync.dma_start(out=out[4], in_=c4[:, :T])
                # ch5 = ch5p * rcnt (per-partition scalar)
                c5 = fin.tile([NM, TC], F32, tag="c5")
                nc.vector.tensor_scalar_mul(c5, ch5p, rcnt_s)
                nc.sync.dma_start(out=out[5], in_=c5[:, :T])

    fix_sync_waits(nc)
    return nc


def _host_prep(inputs):
    wav = np.asarray(inputs["waveform"], dtype=np.float32)          # [8,2,NS]
    W_real = np.asarray(inputs["W_real"], dtype=np.float32)         # [ND,F]
    W_imag = np.asarray(inputs["W_imag"], dtype=np.float32)
    norm_hr = np.asarray(inputs["norm_hr_sq"], dtype=np.float32)
    norm_hl = np.asarray(inputs["norm_hl_sq"], dtype=np.float32)
    az = np.asarray(inputs["az_bin_idx"]).astype(np.int64)          # [ND]
    win = np.asarray(inputs["window"], dtype=np.float32)            # [NFFT]
    mel_fb = np.asarray(inputs["mel_fb"], dtype=np.float32)         # [NM,F]

    xpad = np.pad(wav, ((0, 0), (0, 0), (NFFT // 2, NFFT // 2 + HOP)))  # [8,2,PADNS]
    # host framing: [2, NFFT, TC] per batch, so device frame chunks are
    # partition-contiguous (the strided on-device gather exploded into
    # per-element 4B DMA descriptors)
    sw = np.lib.stride_tricks.sliding_window_view(xpad, NFFT, axis=2)[:, :, ::HOP]
    # sw: [8, 2, TC, NFFT] view -> transpose to [8, 2, NFFT, TC] contiguous
    frames_np = np.ascontiguousarray(np.swapaxes(sw, 2, 3))

    n = np.arange(NFFT, dtype=np.float64)[:, None]
    k = np.arange(F, dtype=np.float64)[None, :]
    ang = 2.0 * np.pi * n * k / NFFT
    dftc = np.zeros((NFFT, FP), dtype=np.float32)
    dfts = np.zeros((NFFT, FP), dtype=np.float32)
    dftc[:, :F] = (np.cos(ang) * win[:, None]).astype(np.float32)
    dfts[:, :F] = (-np.sin(ang) * win[:, None]).astype(np.float32)

    def tile_dft(a):  # [NFFT, FP] -> [NFT, 128, NCH, 128]
        return np.ascontiguousarray(
            a.reshape(NCH, 128, NFT, 128).transpose(2, 1, 0, 3))

    def tile_w(mat):  # [ND, F] -> lhsT tiled [NDT, 128, NFT, 128] bf16
        t = np.zeros((FP, ND), dtype=np.float32)
        t[:F] = mat.T
        return np.ascontiguousarray(
            t.reshape(NFT, 128, NDT, 128).transpose(2, 1, 0, 3)
        ).astype(ml_dtypes.bfloat16)

    wwr = tile_w(W_real)
    wwi = tile_w(-W_imag)
    whr = tile_w(norm_hr)
    whl = tile_w(norm_hl)

    melT = np.zeros((FP, NM), dtype=np.float32)
    melT[:F] = mel_fb.T
    melwf = np.ascontiguousarray(melT.reshape(NFT, 128, NM).transpose(1, 0, 2))
    melw = melwf.astype(ml_dtypes.bfloat16)

    cnt = np.bincount(az, minlength=NM).astype(np.float32)
    S01 = (az[:, None] == np.arange(NM)[None, :]).astype(np.float32)  # [ND,NM]
    sTw = np.ascontiguousarray(S01.reshape(NDT, 128, NM)).astype(ml_dtypes.bfloat16)
    rcnt = (1.0 / (cnt + EPS)).astype(np.float32).reshape(NM, 1)

    shared = {
        "dftc": tile_dft(dftc), "dfts": tile_dft(dfts),
        "wwr": wwr, "wwi": wwi, "whr": whr, "whl": whl,
        "melw": melw, "melwf": melwf, "sTw": sTw, "rcnt": rcnt,
    }
    in_maps = []
    for b in range(B):
        m = dict(shared)
        m["frames"] = frames_np[b]
        in_maps.append(m)
    return in_maps


def kernel(**inputs) -> np.ndarray:
    if "nc" not in _NC_CACHE:
        _NC_CACHE["nc"] = build_nc()
    nc = _NC_CACHE["nc"]
    in_maps = _host_prep(inputs)
    res = run_bass_kernel_spmd(nc, in_maps, core_ids=list(range(B)))
    out = np.stack([np.asarray(res.results[i]["out"]) for i in range(B)])
    return out.astype(np.float32)
